# revision 2
# baseline (speedup 1.0000x reference)
"""Trainium2 Bass kernel for nn_ContactPredictionHead.

Reference computation (B=2, L=2048, D=1536, T=2):
    Wp, Wd = W[:, :D], W[:, D:]
    prod[b,i,j,t] = sum_d h[b,i,d] * Wp[t,d] * h[b,j,d]
    diff[b,i,j,t] = (h@Wd.T)[b,i,t] - (h@Wd.T)[b,j,t]
    out = symmetrize(prod + diff + bias)

Key identity: prod is symmetric in (i,j) and diff is antisymmetric, so the
symmetrization leaves   out[b,i,j,t] = prod[b,i,j,t] + bias[t]   exactly.
That is 4 big matmuls ([2048,1536] @ [1536,2048], one per (b,t) pair).

Sharding: core c in 0..7 handles batch b = c//4 and a 512-row slice of the
i axis.  Each core receives h[b].T (the shared j-axis operand), its own row
slice transposed, and Wp.T; it computes the two weighted Gram blocks
[512, 2048] (t=0,1) and writes them interleaved as [512, 2048, 2].

The matmuls run in float32r (full-rate fp32 on the PE array, ~1e-4 rel err).
"""
import sys

sys.path.insert(0, "/opt/trn_rl_repo")

import numpy as np

B, L, D, T = 2, 2048, 1536, 2
NCORES = 8
CPB = NCORES // B     # cores per batch item = 4
RPC = L // CPB        # output rows per core = 512
NK = D // 128         # contraction k-tiles = 12
NM = RPC // 128       # output row tiles per core = 4
NJ = 512              # j columns per matmul (one PSUM bank of fp32)
NNB = L // NJ         # j blocks = 4

_CACHE = {}


def _get_nc():
    if "nc" in _CACHE:
        return _CACHE["nc"]
    import concourse.tile as tile
    from concourse import bacc, mybir

    f32, f32r = mybir.dt.float32, mybir.dt.float32r
    nc = bacc.Bacc("TRN2", target_bir_lowering=False, debug=False,
                   num_devices=NCORES)
    ht_d = nc.dram_tensor("ht", [D, L], f32r, kind="ExternalInput")
    hrt_d = nc.dram_tensor("hrt", [D, RPC], f32, kind="ExternalInput")
    wp_d = nc.dram_tensor("wp", [D, T], f32, kind="ExternalInput")
    out_d = nc.dram_tensor("out", [RPC, L, T], f32, kind="ExternalOutput")

    with tile.TileContext(nc) as tc:
        with tc.tile_pool(name="big", bufs=1) as big, \
             tc.tile_pool(name="st", bufs=4) as stp, \
             tc.tile_pool(name="ps", bufs=4, space="PSUM") as psp:
            # Per-partition Wp scales: column t*NK+k holds Wp[t, 128k:128k+128].
            wt = big.tile([128, T * NK], f32, name="wt")
            for t in range(T):
                for k in range(NK):
                    nc.scalar.dma_start(wt[:, t * NK + k: t * NK + k + 1],
                                        wp_d[k * 128:(k + 1) * 128, t:t + 1])
            # Row slice, transposed: hr[p, k*RPC + i] = h[b, r0+i, 128k+p].
            hr = big.tile([128, NK * RPC], f32, name="hr")
            nc.scalar.dma_start(
                hr[:].rearrange("p (k i) -> p k i", k=NK),
                hrt_d.ap().rearrange("(k p) i -> p k i", p=128))
            # Stationary operands: a[t][p, k*RPC + i] = hr * Wp[t, 128k+p],
            # rounded to fp32r.
            a = [big.tile([128, NK * RPC], f32r, name=f"a{t}") for t in range(T)]
            for t in range(T):
                for k in range(NK):
                    nc.vector.tensor_scalar_mul(
                        a[t][:, k * RPC:(k + 1) * RPC],
                        hr[:, k * RPC:(k + 1) * RPC],
                        wt[:, t * NK + k: t * NK + k + 1])
            # Moving operand: one 3 MB DMA per j block.
            # htn[n][p, k*NJ + j] = h[b, n*NJ+j, 128k+p].
            htn = [big.tile([128, NK * NJ], f32r, name=f"htn{n}")
                   for n in range(NNB)]
            for n in range(NNB):
                nc.sync.dma_start(
                    htn[n][:].rearrange("p (k j) -> p k j", k=NK),
                    ht_d.ap()[:, n * NJ:(n + 1) * NJ]
                    .rearrange("(k p) j -> p k j", p=128))

            for n in range(NNB):
                for m in range(NM):
                    st = stp.tile([128, NJ * T], f32, name="st", tag="st")
                    for t in range(T):
                        acc = psp.tile([128, NJ], f32, name="acc", tag="acc")
                        for k in range(NK):
                            nc.tensor.matmul(
                                acc[:],
                                a[t][:, k * RPC + m * 128: k * RPC + (m + 1) * 128],
                                htn[n][:, k * NJ:(k + 1) * NJ],
                                start=(k == 0), stop=(k == NK - 1))
                        # Interleave t into the stage tile: st[p, 2j+t].
                        if t == 0:
                            nc.vector.tensor_copy(st[:, t:NJ * T:2], acc[:])
                        else:
                            nc.scalar.copy(st[:, t:NJ * T:2], acc[:])
                    nc.scalar.dma_start(
                        out_d.ap()[m * 128:(m + 1) * 128,
                                   n * NJ:(n + 1) * NJ, :]
                        .rearrange("p j t -> p (j t)"),
                        st[:])
    nc.compile()
    _CACHE["nc"] = nc
    return nc


def kernel(hidden_states, W, b):
    from concourse.bass_utils import run_bass_kernel_spmd

    h = np.ascontiguousarray(hidden_states, dtype=np.float32)
    W = np.asarray(W, dtype=np.float32)
    bias = np.asarray(b, dtype=np.float32)
    nc = _get_nc()

    wp_t = np.ascontiguousarray(W[:, :D].T)          # [D, T]
    hts = [np.ascontiguousarray(h[bi].T) for bi in range(B)]   # [D, L] each
    in_maps = []
    for c in range(NCORES):
        bi, r0 = c // CPB, (c % CPB) * RPC
        in_maps.append({
            "ht": hts[bi],
            "hrt": np.ascontiguousarray(h[bi, r0:r0 + RPC].T),  # [D, RPC]
            "wp": wp_t,
        })
    res = run_bass_kernel_spmd(nc, in_maps, core_ids=list(range(NCORES)))
    out = np.empty((B, L, L, T), np.float32)
    for c in range(NCORES):
        bi, r0 = c // CPB, (c % CPB) * RPC
        out[bi, r0:r0 + RPC] = res.results[c]["out"]
    if np.any(bias != 0):
        out += bias
    return out
